# revision 1
# baseline (speedup 1.0000x reference)
"""MultiHeadAttention (faithful raw-reshape variant) on 8 trn2 NeuronCores.

Math (per batch b):
  Y  = Xq @ Wq.T            [S, D]
  Z  = Xk @ Wk.T            [S, D]
  V  = Xv @ Wv.T            [S, D]
  reshape (B,S,D)->(B,H,S,dk) is a *raw view*: head h <- rows [128h, 128h+128)
  of Y/Z/V; within the block, q = 16t + j maps to (row t, features 64j..64j+64).
  A  = softmax(Qh @ Kh.T / 8), O = A @ Vh, placed back into the same raw view,
  out = Hcat @ Wo.T + b_o.

Because heads partition the *rows* of Y/Z/V, the computation is fully
independent across (b, h): 32 tasks, 4 per core, no collectives.

Per-core device program (heads hl=0..3 over the core's 512 rows):
  QT/KT: transposed projections  QT[f, s] (f on partitions, 8x128 chunks)
  V:     normal orientation with a 32-wide ones block -> [128, 16, 96]
  scores (transposed): AT[t', q] = sum_k KT[64j'+k, t'] QT[64j+k, t];
         query blocks of opposite j-parity run as pairs on PE row groups
         0-63 / 64-127 (row tiling -> concurrent, 2x score throughput)
  exp on ACT with scale=1/8 fused; no max subtraction (scores ~N(0,1), fp32
         exp cannot overflow); one ACTIVATE per PSUM bank (2-bank reads hang)
  AV:    O.T[k', q] += V[:, j', :].T @ expAT ; rows 64-95 = denominator copies
  normalize: DVE reciprocal + 32-wide quadrant-aligned multiplies into HcatT
  out:   out[t, f'] = HcatT.T @ WoT + b_o

Fallback: TRN_MM_DTYPE=f32 env selects exact fp32 matmuls (~1.5x slower).
"""

import os

import numpy as np

import concourse.bass as bass
import concourse.mybir as mybir
import concourse.tile as tile
from concourse import bacc

B, S, D = 2, 2048, 1024
H, DK = 16, 64
NCORES = 8
HPC = H // (NCORES // B)  # heads per core = 4
SC = HPC * 128            # s-rows per core = 512
P = 128
KD = D // P               # 8 contraction chunks
PO = D // P               # 8 feature chunks
F32 = mybir.dt.float32

# matmul dtype mode: "f32" | "f32r" | "bf16"
MODE = os.environ.get("TRN_MM_DTYPE", "bf16")


def _mm_dt():
    return {"f32": mybir.dt.float32, "f32r": mybir.dt.float32,
            "bf16": mybir.dt.bfloat16}[MODE]


def _np_dt():
    import ml_dtypes
    return {"f32": np.float32, "f32r": np.float32,
            "bf16": ml_dtypes.bfloat16}[MODE]


def _c(ap):
    """Bitcast matmul operands to float32r in f32r mode."""
    if MODE == "f32r":
        return ap.bitcast(mybir.dt.float32r)
    return ap


def build_body(nc, out_ap, ins):
    """Emit the per-core program. ins: dict of DRAM APs."""
    xqt, xkt, xvt = ins["xqt"], ins["xkt"], ins["xvt"]
    wqt, wkt, wvt, wot = ins["wqt"], ins["wkt"], ins["wvt"], ins["wot"]
    bo = ins["bo"]
    mdt = _mm_dt()
    EXPF = mybir.ActivationFunctionType.Exp
    MULT = mybir.AluOpType.mult
    ADD = mybir.AluOpType.add

    wide = MODE == "bf16"  # fp32 storage doesn't fit double-buffered weights
    with tile.TileContext(nc) as tc:
        with (
            tc.tile_pool(name="singles", bufs=1) as singles,
            tc.tile_pool(name="wp", bufs=2 if wide else 1) as wp,
            tc.tile_pool(name="xp", bufs=2) as xp,
            tc.tile_pool(name="exp", bufs=10 if wide else 4) as exp_pool,
            tc.tile_pool(name="smalls", bufs=3) as smalls,
            tc.tile_pool(name="outp", bufs=4) as outp,
            tc.tile_pool(name="ps_mm", bufs=2, space="PSUM") as ps_mm,
            tc.tile_pool(name="ps_at", bufs=4, space="PSUM") as ps_at,
            tc.tile_pool(name="ps_o", bufs=2, space="PSUM") as ps_o,
        ):
            # --- constants ---
            bo_sb = singles.tile([P, D], F32, tag="bo", name="bo_sb")
            bo_bcast = bass.AP(tensor=bo.tensor, offset=bo.offset,
                               ap=[[0, P], list(bo.ap[-1])])
            nc.gpsimd.dma_start(out=bo_sb, in_=bo_bcast)

            qt_sb = singles.tile([P, PO, SC], mdt, tag="qt", name="qt_sb")
            kt_sb = singles.tile([P, PO, SC], mdt, tag="kt", name="kt_sb")
            # partition-rotated copy: kt2[pi] = kt[(pi+64) % 128], so a key
            # slice of either j'-parity is available at either partition base
            kt2_sb = singles.tile([P, PO, SC], mdt, tag="kt2", name="kt2_sb")
            hcat = singles.tile([P, PO, SC], mdt, tag="hcat", name="hcat")
            # [V | ones*32]: the A@V matmul then emits 32 copies of the
            # softmax denominator on partitions 64..95 (time is free: matmul
            # cost depends only on the moving-operand free size)
            v_sb = [singles.tile([P, 16, 96], mdt, tag=f"v{hl}", name=f"v_sb{hl}")
                    for hl in range(HPC)]
            for hl in range(HPC):
                nc.vector.memset(v_sb[hl][:, :, 64:96], 1.0)

            # --- projections ---
            # loads split by kd quarters: region-level deps let the first
            # projection matmuls start after 1/4 of the tensor lands
            def load_w(ap):
                t = wp.tile([P, KD, D], mdt, tag="w", name="w_t")
                src = ap.rearrange("(kd p) f -> p kd f", p=P)
                for q in range(0, KD, 2):
                    nc.sync.dma_start(t[:, q:q + 2], src[:, q:q + 2])
                return t

            def load_x(ap):
                t = xp.tile([P, KD, SC], mdt, tag="x", name="x_t")
                src = ap.rearrange("(kd p) s -> p kd s", p=P)
                for q in range(0, KD, 2):
                    nc.sync.dma_start(t[:, q:q + 2], src[:, q:q + 2])
                return t

            # Q/K transposed: QT[f, s] = sum_d WqT[d, f] XqT[d, s]
            for w_ap, x_ap, dst in ((wqt, xqt, qt_sb), (wkt, xkt, kt_sb)):
                w_t, x_t = load_w(w_ap), load_x(x_ap)
                for mf in range(PO):
                    ps = ps_mm.tile([P, SC], F32, tag="mm", name="ps")
                    for kd in range(KD):
                        nc.tensor.matmul(
                            ps, _c(w_t[:, kd, mf * P:(mf + 1) * P]),
                            _c(x_t[:, kd, :]),
                            start=(kd == 0), stop=(kd == KD - 1))
                    nc.vector.tensor_copy(dst[:, mf, :], ps)
                    if dst is kt_sb:
                        nc.sync.dma_start(kt2_sb[0:64, mf],
                                          kt_sb[64:128, mf])
                        nc.sync.dma_start(kt2_sb[64:128, mf],
                                          kt_sb[0:64, mf])

            # V normal: V[s, f] = sum_d XvT[d, s] WvT[d, f].
            # Only head 0's V up front; heads 1-3 are emitted after head 0's
            # first attention half so PE feeds ACT scores sooner (emission
            # order is dependency order in Tile, so v_proj(h) must still
            # precede head h's first A@V matmul).
            wv_t, xv_t = load_w(wvt), load_x(xvt)

            def v_proj(hl):
                for nf in range(2):
                    ps = ps_mm.tile([P, SC], F32, tag="mm", name="ps")
                    for kd in range(KD):
                        nc.tensor.matmul(
                            ps, _c(xv_t[:, kd, hl * P:(hl + 1) * P]),
                            _c(wv_t[:, kd, nf * 512:(nf + 1) * 512]),
                            start=(kd == 0), stop=(kd == KD - 1))
                    nc.vector.tensor_copy(
                        v_sb[hl][:, nf * 8:(nf + 1) * 8, 0:64],
                        ps.rearrange("p (j k) -> p j k", k=64))

            v_proj(0)

            wo_t = load_w(wot)  # prefetched during attention

            # --- attention per local head ---
            # Query blocks of opposite j-parity are processed in pairs: their
            # score matmuls run on PE row-groups 0-63 / 64-127 and execute
            # concurrently (row tiling), doubling score throughput.
            # outproj(h) is emitted after head h+1's first half so the next
            # head's scores reach PE at the boundary and ACT never starves.
            pending = []
            for hl in range(HPC):
                hs = slice(hl * P, (hl + 1) * P)
                for pp in range(2):  # po-half; qbA has a=0, qbB a=1
                    rhs_a = qt_sb[0:64, 4 * pp:4 * pp + 4, hs]
                    rhs_b = qt_sb[64:128, 4 * pp:4 * pp + 4, hs]
                    o_a = ps_o.tile([96, 512], F32, tag="o", name="o_a")
                    o_b = ps_o.tile([96, 512], F32, tag="o", name="o_b")
                    for jp in range(16):
                        a2, po2 = jp % 2, jp // 2
                        ksrc_a = kt_sb if a2 == 0 else kt2_sb
                        ksrc_b = kt_sb if a2 == 1 else kt2_sb
                        at_a = ps_at.tile([P, 512], F32, tag="at", name="at_a")
                        at_b = ps_at.tile([P, 512], F32, tag="at", name="at_b")
                        # adjacent matmuls on PE row groups 0-63 / 64-127
                        # execute concurrently (row tiling)
                        nc.tensor.matmul(at_a, _c(ksrc_a[0:64, po2, hs]),
                                         _c(rhs_a), start=True, stop=True)
                        nc.tensor.matmul(at_b, _c(ksrc_b[64:128, po2, hs]),
                                         _c(rhs_b), start=True, stop=True)
                        ex_a = exp_pool.tile([P, 512], mdt, tag="ex",
                                             name="ex_a")
                        ex_b = exp_pool.tile([P, 512], mdt, tag="ex",
                                             name="ex_b")
                        # note: a single ACTIVATE must not read >1 PSUM bank
                        # (2-bank reads hang the device)
                        nc.scalar.activation(ex_a, at_a, EXPF, scale=0.125)
                        nc.scalar.activation(ex_b, at_b, EXPF, scale=0.125)
                        st, sp = jp == 0, jp == 15
                        nc.tensor.matmul(o_a, _c(v_sb[hl][:, jp, :]),
                                         _c(ex_a), start=st, stop=sp)
                        nc.tensor.matmul(o_b, _c(v_sb[hl][:, jp, :]),
                                         _c(ex_b), start=st, stop=sp)
                    # normalize into HcatT: recip of the replicated denom
                    # rows, then 32-wide multiplies (quadrant-aligned)
                    for a, o_ps in ((0, o_a), (1, o_b)):
                        rc = smalls.tile([P, 512], F32, tag="rc", name="rc")
                        nc.vector.reciprocal(rc[64:96, :], o_ps[64:96, :])
                        dst = hcat[64 * a:64 * a + 64, 4 * pp:4 * pp + 4, hs]
                        for u in range(2):
                            nc.vector.tensor_tensor(
                                dst[32 * u:32 * u + 32],
                                o_ps[32 * u:32 * u + 32, :].rearrange(
                                    "k (c t) -> k c t", t=P),
                                rc[64:96, :].rearrange("k (c t) -> k c t", t=P),
                                MULT)

                    if pp == 0:
                        if hl == 0:
                            for h2 in range(1, HPC):
                                v_proj(h2)
                        for emit in pending:
                            emit()
                        pending = []

                # output projection for this head block (deferred emission)
                def outproj(hs=hs):
                    for nf in range(2):
                        fs = slice(nf * 512, (nf + 1) * 512)
                        ps = ps_mm.tile([P, 512], F32, tag="mm", name="ps")
                        for po in range(PO):
                            nc.tensor.matmul(
                                ps, _c(hcat[:, po, hs]), _c(wo_t[:, po, fs]),
                                start=(po == 0), stop=(po == PO - 1))
                        os_t = outp.tile([P, 512], F32, tag="os", name="os_t")
                        nc.vector.tensor_tensor(os_t, ps, bo_sb[:, fs], ADD)
                        nc.sync.dma_start(out_ap[hs, fs], os_t)
                pending.append(outproj)

            for emit in pending:
                emit()
    return nc


def build_program():
    nc = bacc.Bacc("TRN2", target_bir_lowering=False, debug=False,
                   enable_asserts=False, num_devices=NCORES)
    mdt = _mm_dt()
    ins = {
        "xqt": nc.dram_tensor("xqt", [D, SC], mdt, kind="ExternalInput").ap(),
        "xkt": nc.dram_tensor("xkt", [D, SC], mdt, kind="ExternalInput").ap(),
        "xvt": nc.dram_tensor("xvt", [D, SC], mdt, kind="ExternalInput").ap(),
        "wqt": nc.dram_tensor("wqt", [D, D], mdt, kind="ExternalInput").ap(),
        "wkt": nc.dram_tensor("wkt", [D, D], mdt, kind="ExternalInput").ap(),
        "wvt": nc.dram_tensor("wvt", [D, D], mdt, kind="ExternalInput").ap(),
        "wot": nc.dram_tensor("wot", [D, D], mdt, kind="ExternalInput").ap(),
        "bo": nc.dram_tensor("bo", [1, D], F32, kind="ExternalInput").ap(),
    }
    out_ap = nc.dram_tensor("out", [SC, D], F32, kind="ExternalOutput").ap()
    build_body(nc, out_ap, ins)
    nc.finalize()
    return nc


def make_in_maps(inputs):
    ndt = _np_dt()
    Xq = np.asarray(inputs["X_q"], dtype=np.float32)
    Xk = np.asarray(inputs["X_k"], dtype=np.float32)
    Xv = np.asarray(inputs["X_v"], dtype=np.float32)
    wqt = np.ascontiguousarray(np.asarray(inputs["W_q"], np.float32).T).astype(ndt)
    wkt = np.ascontiguousarray(np.asarray(inputs["W_k"], np.float32).T).astype(ndt)
    wvt = np.ascontiguousarray(np.asarray(inputs["W_v"], np.float32).T).astype(ndt)
    wot = np.ascontiguousarray(np.asarray(inputs["W_o"], np.float32).T).astype(ndt)
    bo = np.asarray(inputs["b_o"], np.float32).reshape(1, D)
    xt = {n: [np.ascontiguousarray(x[b].T).astype(ndt) for b in range(B)]
          for n, x in (("xqt", Xq), ("xkt", Xk), ("xvt", Xv))}
    in_maps = []
    for c in range(NCORES):
        b, g = divmod(c, NCORES // B)
        sl = slice(g * SC, (g + 1) * SC)
        in_maps.append({
            "xqt": np.ascontiguousarray(xt["xqt"][b][:, sl]),
            "xkt": np.ascontiguousarray(xt["xkt"][b][:, sl]),
            "xvt": np.ascontiguousarray(xt["xvt"][b][:, sl]),
            "wqt": wqt, "wkt": wkt, "wvt": wvt, "wot": wot, "bo": bo,
        })
    return in_maps


_NC_CACHE = {}


def _run(inputs, trace=False, trace_cores=None):
    from concourse.bass_utils import run_bass_kernel_spmd
    if MODE not in _NC_CACHE:
        _NC_CACHE[MODE] = build_program()
    nc = _NC_CACHE[MODE]
    in_maps = make_in_maps(inputs)
    res = run_bass_kernel_spmd(nc, in_maps, core_ids=list(range(NCORES)),
                               trace=trace, trace_cores=trace_cores)
    out = np.empty((B, S, D), dtype=np.float32)
    for c in range(NCORES):
        b, g = divmod(c, NCORES // B)
        out[b, g * SC:(g + 1) * SC, :] = res.results[c]["out"]
    return out, res


def kernel(**inputs):
    out, _ = _run(inputs, trace=False)
    return out

